# revision 9
# baseline (speedup 1.0000x reference)
"""GCN (2-layer GraphConv + classifier) on 8 Trainium2 NeuronCores.

Strategy: shard nodes (and their in-edges) across the 8 cores via a
load-balancing permutation; replicate the small weights; zero-copy AllGather
(Shared-output) publishes per-layer node features; segment-sum via one-hot
M-matmuls on the tensor engine with edge rows fetched by SWDGE dma_gather
spread over 4 queues. Layer 2 aggregates h1s = relu(agg1*inv_d+b1)*inv_s
(384 wide) and folds (W2 @ Wc) after aggregation, so both layers share one
768B-row gather structure and the narrow-z path disappears. One-hot M tiles
are generated on-chip (iota/is_equal) instead of streamed from HBM.
"""
import os
import sys

sys.path.insert(0, "/opt/trn_rl_repo")

import numpy as np
import ml_dtypes

import concourse.bacc as bacc
import concourse.bass as bass
import concourse.mybir as mybir
import concourse.tile as tile
from concourse import library_config
from concourse.masks import make_identity

NCORES = 8
P = 128
N_NODES = 50000
N_EDGES = 400000
NP_PAD = 50176            # 8 * 6272
R = NP_PAD // NCORES      # 6272 rows per core
RT = R // P               # 49 row tiles per core
HALF = NP_PAD // 2        # 25088 (< 32768 so int16 indices work per half)
IN_F = 1433
KP = 1536                 # padded contraction (12 * 128)
KC = KP // P              # 12 k-chunks
HID = 384
N_CLS = 7
GROUP_TILES = 4           # dst tiles per gather-call group
NSWQ = 4                  # SWDGE queues for gathers

bf16 = ml_dtypes.bfloat16


def _balance_nodes(edge_src, edge_dst):
    """Permute nodes -> slots so that per-(tile, src-half) edge buckets are
    near-equal, minimizing 128-padding and the max-over-cores chunk counts.

    Returns perm (node id -> slot)."""
    src = edge_src.astype(np.int64)
    dst = edge_dst.astype(np.int64)
    deg_out = np.bincount(src, minlength=NP_PAD).astype(np.int64)

    # --- step 1: split nodes into two halves balancing out-degree ---
    order = np.argsort(-deg_out, kind="stable")
    half_of = np.zeros(NP_PAD, np.int8)
    sums = [0, 0]
    counts = [0, 0]
    cap = HALF
    for n in order:
        h = 0 if (sums[0] <= sums[1] and counts[0] < cap) or counts[1] >= cap else 1
        half_of[n] = h
        sums[h] += deg_out[n]
        counts[h] += 1

    # --- per-node in-degree split by src half ---
    in_lo = np.bincount(dst[half_of[src] == 0], minlength=NP_PAD).astype(np.int64)
    in_hi = np.bincount(dst[half_of[src] == 1], minlength=NP_PAD).astype(np.int64)

    # --- step 2: pack nodes of each half into 196 dst bins of 128 slots,
    # balancing (in_lo, in_hi) jointly ---
    NB = NCORES * RT // 2     # bins per half
    bins_of_half = {}
    for h in (0, 1):
        nodes = np.nonzero(half_of == h)[0]
        keys = np.maximum(in_lo[nodes], in_hi[nodes])
        nodes = nodes[np.argsort(-keys, kind="stable")]
        bsum = np.zeros((NB, 2), np.int64)
        bcnt = np.zeros(NB, np.int64)
        assign = np.empty(len(nodes), np.int64)
        big = np.iinfo(np.int64).max
        for i in range(len(nodes)):
            n = nodes[i]
            l, hh = in_lo[n], in_hi[n]
            cost = np.maximum(bsum[:, 0] + l, bsum[:, 1] + hh)
            cost = np.where(bcnt < P, cost, big)
            b = int(np.argmin(cost))
            assign[i] = b
            bsum[b, 0] += l
            bsum[b, 1] += hh
            bcnt[b] += 1
        bins_of_half[h] = ([nodes[assign == b] for b in range(NB)], bsum)

    # --- step 3: pair bins into tiles, grouping similar sizes to shrink the
    # max-over-cores ---
    perm = np.empty(NP_PAD, np.int64)
    for h in (0, 1):
        bins, bsum = bins_of_half[h]
        rank = np.argsort(-(bsum[:, 0] * 100000 + bsum[:, 1]), kind="stable")
        cores = range(0, 4) if h == 0 else range(4, 8)
        k = 0
        for t in range(RT):
            for c in cores:
                nodes = bins[rank[k]]
                k += 1
                assert len(nodes) == P
                perm[nodes] = c * R + t * P + np.arange(P)
    return perm


def _build_edge_plan(edge_src, edge_dst):
    """Permute nodes for balance, partition edges by dst core, per (core,
    tile) split by src half; pad to 128-edge chunks with a uniform
    (max-over-cores) per-(tile, half) chunk count."""
    src0 = edge_src.astype(np.int64)
    dst0 = edge_dst.astype(np.int64)
    perm = _balance_nodes(src0, dst0)
    src = perm[src0]
    dst = perm[dst0]

    core = dst // R
    t_all = (dst % R) // P
    p_all = dst % P
    half_all = (src >= HALF).astype(np.int64)
    src_rel = src - HALF * half_all

    counts = np.zeros((NCORES, RT, 2), np.int64)
    np.add.at(counts, (core, t_all, half_all), 1)
    chunks = np.maximum(np.ceil(counts / P).astype(np.int64).max(axis=0), 1)
    chunks_lo = chunks[:, 0]
    chunks_hi = chunks[:, 1]

    n_groups = (RT + GROUP_TILES - 1) // GROUP_TILES
    groups = []
    chunk_base = 0
    lo_off = np.zeros(RT, np.int64)
    hi_off = np.zeros(RT, np.int64)
    for g in range(n_groups):
        tiles = list(range(g * GROUP_TILES, min((g + 1) * GROUP_TILES, RT)))
        lo_tot = int(chunks_lo[tiles].sum())
        hi_tot = int(chunks_hi[tiles].sum())
        ofs = chunk_base
        for t in tiles:
            lo_off[t] = ofs
            ofs += chunks_lo[t]
        for t in tiles:
            hi_off[t] = ofs
            ofs += chunks_hi[t]
        groups.append((tiles, lo_tot, hi_tot, chunk_base))
        chunk_base += lo_tot + hi_tot
    c_tot = chunk_base

    order = np.lexsort((src_rel, half_all, t_all, core))
    idx_all = np.zeros((NCORES, c_tot * P), np.int64)
    pcol_all = np.full((NCORES, c_tot * P), -1.0, np.float32)

    srt_core = core[order]
    srt_t = t_all[order]
    srt_half = half_all[order]
    srt_src = src_rel[order]
    srt_p = p_all[order]

    core_starts = np.searchsorted(srt_core, np.arange(NCORES + 1))
    for c in range(NCORES):
        s, e = core_starts[c], core_starts[c + 1]
        tt = srt_t[s:e]
        hh = srt_half[s:e]
        ss = srt_src[s:e]
        pp = srt_p[s:e]
        key = tt * 2 + hh
        if len(key):
            new_run = np.concatenate([[True], key[1:] != key[:-1]])
            run_ids = np.cumsum(new_run) - 1
            first_pos = np.nonzero(new_run)[0]
            run_start = first_pos[run_ids]
            pos_in_run = np.arange(len(key)) - run_start
            base = np.where(hh == 0, lo_off[tt], hi_off[tt]) * P
            gpos = base + pos_in_run
            idx_all[c][gpos] = ss
            pcol_all[c][gpos] = pp

    # p-column table for on-chip one-hot gen: [P(slot in chunk), c_tot]
    pcol = np.ascontiguousarray(
        pcol_all.reshape(NCORES, c_tot, P).transpose(0, 2, 1)).astype(np.float32)

    idx_wrapped = np.zeros((NCORES, P, c_tot * P // 16), np.int16)
    for c in range(NCORES):
        w = idx_all[c].astype(np.int16).reshape(-1, 16).T
        idx_wrapped[c] = np.tile(w, (8, 1))

    return dict(
        chunks_lo=chunks_lo, chunks_hi=chunks_hi, groups=groups, c_tot=c_tot,
        pcol=pcol, idx_wrapped=idx_wrapped, perm=perm,
    )


ALL_PHASES = frozenset(["p1", "ag1", "g1", "mm1", "ag2", "g2", "mm2", "fin"])


def _build_nc(plan, repeat=1, phases=None, swq=NSWQ):
    on = ALL_PHASES if phases is None else frozenset(phases)
    chunks_lo = plan["chunks_lo"]
    chunks_hi = plan["chunks_hi"]
    groups = plan["groups"]
    c_tot = plan["c_tot"]

    nc = bacc.Bacc("TRN2", target_bir_lowering=False, debug=False,
                   num_devices=NCORES, num_swdge_queues=swq)
    dt = mybir.dt

    # ---- I/O ----
    xT = nc.dram_tensor("xT", [RT, P, KC * P], dt.bfloat16, kind="ExternalInput")
    w1 = nc.dram_tensor("w1", [P, KC * HID], dt.bfloat16, kind="ExternalInput")
    w2c = nc.dram_tensor("w2c", [P, 3 * 8], dt.bfloat16, kind="ExternalInput")
    b1t = nc.dram_tensor("b1t", [P, HID], dt.float32, kind="ExternalInput")
    bct = nc.dram_tensor("bct", [P, 8], dt.float32, kind="ExternalInput")
    inv_s_t = nc.dram_tensor("inv_s_t", [P, RT], dt.float32, kind="ExternalInput")
    inv_d_t = nc.dram_tensor("inv_d_t", [P, RT], dt.float32, kind="ExternalInput")
    pcol_in = nc.dram_tensor("pcol", [P, c_tot], dt.float32, kind="ExternalInput")
    iota_in = nc.dram_tensor("iota", [P, P], dt.bfloat16, kind="ExternalInput")
    idxs = nc.dram_tensor("idxs", [P, c_tot * P // 16], dt.int16, kind="ExternalInput")
    out = nc.dram_tensor("out", [P, RT * N_CLS], dt.float32, kind="ExternalOutput")

    # ---- internal DRAM ----
    h_c = nc.dram_tensor("h_c", [R, HID], dt.bfloat16)
    h_full = nc.dram_tensor("h_full", [NP_PAD, HID], dt.bfloat16, addr_space="Shared")
    h1_c = nc.dram_tensor("h1_c", [R, HID], dt.bfloat16)
    h1_full = nc.dram_tensor("h1_full", [NP_PAD, HID], dt.bfloat16, addr_space="Shared")

    rg = [list(range(NCORES))]

    with tile.TileContext(nc) as tc:
        with (
            tc.tile_pool(name="const", bufs=1) as const,
            tc.tile_pool(name="xload", bufs=3) as xload,
            tc.tile_pool(name="hout", bufs=3) as hout,
            tc.tile_pool(name="glo", bufs=2) as glo,
            tc.tile_pool(name="ghi", bufs=2) as ghi,
            tc.tile_pool(name="mgen", bufs=2) as mgen,
            tc.tile_pool(name="work", bufs=4) as work,
            tc.tile_pool(name="psA", bufs=2, space="PSUM") as psA,
            tc.tile_pool(name="psB", bufs=2, space="PSUM") as psB,
        ):
            nc.gpsimd.load_library(library_config.mlp)
            qctr = [0]

            def nextq():
                q = qctr[0] % swq
                qctr[0] += 1
                return q

            w1_t = const.tile([P, KC * HID], dt.bfloat16)
            nc.sync.dma_start(out=w1_t[:], in_=w1[:])
            w2c_t = const.tile([P, 3 * 8], dt.bfloat16)
            nc.sync.dma_start(out=w2c_t[:], in_=w2c[:])
            b1_t = const.tile([P, HID], dt.float32)
            nc.sync.dma_start(out=b1_t[:], in_=b1t[:])
            bc_t = const.tile([P, 8], dt.float32)
            nc.sync.dma_start(out=bc_t[:], in_=bct[:])
            invs_t = const.tile([P, RT], dt.float32)
            nc.sync.dma_start(out=invs_t[:], in_=inv_s_t[:])
            invd_t = const.tile([P, RT], dt.float32)
            nc.sync.dma_start(out=invd_t[:], in_=inv_d_t[:])
            pcol_t = const.tile([P, c_tot], dt.float32)
            nc.sync.dma_start(out=pcol_t[:], in_=pcol_in[:])
            iota_t = const.tile([P, P], dt.bfloat16)
            nc.sync.dma_start(out=iota_t[:], in_=iota_in[:])
            idx_t = const.tile([P, c_tot * P // 16], dt.int16)
            nc.sync.dma_start(out=idx_t[:], in_=idxs[:])
            ident = const.tile([P, P], dt.bfloat16)
            make_identity(nc, ident[:])
            dummy = const.tile([P, HID], dt.bfloat16)
            nc.vector.memset(dummy[:], 0.0)

            out_t = const.tile([P, RT * N_CLS], dt.float32)

            def agg_layer(layer, src_tab, dst_dram):
                """One gather+aggregate pass over all edge groups.

                layer==1: h1s = relu(agg*inv_d + b1)*inv_s -> dst_dram rows
                layer==2: out_t[:, t*7:(t+1)*7] = (agg*inv_d) @ W2c + bc
                """
                gk = "g1" if layer == 1 else "g2"
                mmk = "mm1" if layer == 1 else "mm2"
                s_lo = src_tab[0:HALF, :]
                s_hi = src_tab[HALF:NP_PAD, :]
                for (tiles, lo_tot, hi_tot, cbase) in groups:
                    nlo = lo_tot * P
                    nhi = hi_tot * P
                    glo_t = glo.tile([P, lo_tot, HID], dt.bfloat16, tag="glo")
                    ghi_t = ghi.tile([P, hi_tot, HID], dt.bfloat16, tag="ghi")
                    if gk in on:
                        nc.gpsimd.dma_gather(
                            out_ap=glo_t[:], in_ap=s_lo,
                            idxs_ap=idx_t[:, cbase * 8:(cbase + lo_tot) * 8],
                            num_idxs=nlo, num_idxs_reg=nlo, elem_size=HID,
                            single_packet=False, queue_num=nextq(),
                        )
                        nc.gpsimd.dma_gather(
                            out_ap=ghi_t[:], in_ap=s_hi,
                            idxs_ap=idx_t[:, (cbase + lo_tot) * 8:(cbase + lo_tot + hi_tot) * 8],
                            num_idxs=nhi, num_idxs_reg=nhi, elem_size=HID,
                            single_packet=False, queue_num=nextq(),
                        )
                    m_t = mgen.tile([P, (lo_tot + hi_tot) * P], dt.bfloat16, tag="m")
                    if mmk in on:
                        # on-chip one-hot: m[e, d] = (iota[d] == pcol[e, chunk])
                        for j in range(lo_tot + hi_tot):
                            nc.vector.tensor_scalar(
                                out=m_t[:, j * P:(j + 1) * P], in0=iota_t[:],
                                scalar1=pcol_t[:, cbase + j:cbase + j + 1],
                                scalar2=None,
                                op0=mybir.AluOpType.is_equal,
                            )
                    lo_pos = 0
                    hi_pos = 0
                    for t in (tiles if mmk in on else []):
                        ncl = int(chunks_lo[t])
                        nch = int(chunks_hi[t])
                        ps = psA.tile([P, HID], dt.float32, space="PSUM", tag="ps")
                        for j in range(ncl):
                            mcol = (lo_pos + j) * P
                            nc.tensor.matmul(
                                out=ps[:], lhsT=m_t[:, mcol:mcol + P],
                                rhs=glo_t[:, lo_pos + j, :] if gk in on else dummy[:],
                                start=(j == 0), stop=False,
                            )
                        for j in range(nch):
                            mcol = (lo_tot + hi_pos + j) * P
                            nc.tensor.matmul(
                                out=ps[:], lhsT=m_t[:, mcol:mcol + P],
                                rhs=ghi_t[:, hi_pos + j, :] if gk in on else dummy[:],
                                start=False, stop=(j == nch - 1),
                            )
                        lo_pos += ncl
                        hi_pos += nch
                        if layer == 1:
                            # h1s = relu(agg*inv_d + b1) * inv_s
                            tmp = work.tile([P, HID], dt.float32, tag="tmp1")
                            nc.vector.scalar_tensor_tensor(
                                out=tmp[:], in0=ps[:], scalar=invd_t[:, t:t + 1],
                                in1=b1_t[:],
                                op0=mybir.AluOpType.mult, op1=mybir.AluOpType.add,
                            )
                            h1t = work.tile([P, HID], dt.bfloat16, tag="h1t")
                            nc.vector.tensor_scalar(
                                out=h1t[:], in0=tmp[:],
                                scalar1=0.0, scalar2=invs_t[:, t:t + 1],
                                op0=mybir.AluOpType.max,
                                op1=mybir.AluOpType.mult,
                            )
                            nc.sync.dma_start(
                                out=dst_dram[t * P:(t + 1) * P, :], in_=h1t[:])
                        else:
                            # out = (agg*inv_d) @ W2c + bc  (transpose agg first)
                            sc = work.tile([P, HID], dt.bfloat16, tag="sc")
                            nc.scalar.activation(
                                out=sc[:], in_=ps[:],
                                func=mybir.ActivationFunctionType.Copy,
                                scale=invd_t[:, t:t + 1],
                            )
                            aggT = work.tile([P, HID], dt.bfloat16, tag="aggT")
                            for k in range(3):
                                pst = psB.tile([P, P], dt.bfloat16, space="PSUM", tag="pst")
                                nc.tensor.transpose(
                                    out=pst[:], in_=sc[:, k * P:(k + 1) * P],
                                    identity=ident[:],
                                )
                                nc.vector.tensor_copy(
                                    out=aggT[:, k * P:(k + 1) * P], in_=pst[:])
                            psz = psB.tile([P, 8], dt.float32, space="PSUM", tag="psz")
                            for k in range(3):
                                nc.tensor.matmul(
                                    out=psz[:],
                                    lhsT=aggT[:, k * P:(k + 1) * P],
                                    rhs=w2c_t[:, k * 8:(k + 1) * 8],
                                    start=(k == 0), stop=(k == 2),
                                )
                            nc.vector.tensor_tensor(
                                out=out_t[:, t * N_CLS:(t + 1) * N_CLS],
                                in0=psz[:, 0:N_CLS], in1=bc_t[:, 0:N_CLS],
                                op=mybir.AluOpType.add,
                            )

            for _rep in range(repeat):
                # ---- Phase 1: h = (x @ W1) * inv_s ----
                if "p1" in on:
                    for r in range(RT):
                        xt = xload.tile([P, KC * P], dt.bfloat16)
                        nc.sync.dma_start(out=xt[:], in_=xT[r])
                        ps = psA.tile([P, HID], dt.float32, space="PSUM")
                        for k in range(KC):
                            nc.tensor.matmul(
                                out=ps[:],
                                lhsT=xt[:, k * P:(k + 1) * P],
                                rhs=w1_t[:, k * HID:(k + 1) * HID],
                                start=(k == 0),
                                stop=(k == KC - 1),
                            )
                        ht = hout.tile([P, HID], dt.bfloat16)
                        nc.scalar.activation(
                            out=ht[:], in_=ps[:],
                            func=mybir.ActivationFunctionType.Copy,
                            scale=invs_t[:, r:r + 1],
                        )
                        nc.sync.dma_start(out=h_c[r * P:(r + 1) * P, :], in_=ht[:])

                # ---- Phase 2: publish h (zero-copy AllGather = barrier) ----
                if "ag1" in on:
                    nc.gpsimd.collective_compute(
                        "AllGather", mybir.AluOpType.bypass, replica_groups=rg,
                        ins=[h_c[:]], outs=[h_full[:]],
                    )

                # ---- Phase 3: layer-1 aggregation -> h1s ----
                agg_layer(1, h_full, h1_c)

                # ---- Phase 4: publish h1s ----
                if "ag2" in on:
                    nc.gpsimd.collective_compute(
                        "AllGather", mybir.AluOpType.bypass, replica_groups=rg,
                        ins=[h1_c[:]], outs=[h1_full[:]],
                    )

                # ---- Phase 5: layer-2 aggregation -> logits ----
                agg_layer(2, h1_full, None)

                if "fin" in on:
                    nc.sync.dma_start(out=out[:], in_=out_t[:])

    nc.compile()
    return nc


def _prepare(features, edge_src, edge_dst, W1, b1, W2, b2, Wc, bc):
    deg_out = np.bincount(edge_src, minlength=N_NODES).astype(np.float32)
    deg_in = np.bincount(edge_dst, minlength=N_NODES).astype(np.float32)
    inv_s = 1.0 / np.sqrt(np.maximum(deg_out, 1.0))
    inv_d = 1.0 / np.sqrt(np.maximum(deg_in, 1.0))
    inv_s = np.concatenate([inv_s, np.ones(NP_PAD - N_NODES, np.float32)])
    inv_d = np.concatenate([inv_d, np.ones(NP_PAD - N_NODES, np.float32)])

    plan = _build_edge_plan(edge_src, edge_dst)
    perm = plan["perm"]

    W1p = np.zeros((KP, HID), np.float32)
    W1p[:IN_F] = W1
    W1p = W1p.astype(bf16)
    w1_sw = np.concatenate([W1p[k * P:(k + 1) * P] for k in range(KC)], axis=1)
    W2c = (W2.astype(np.float32) @ Wc.astype(np.float32))
    W2cp = np.zeros((HID, 8), np.float32)
    W2cp[:, :N_CLS] = W2c
    W2cp16 = W2cp.astype(bf16)
    w2c_sw = np.concatenate([W2cp16[k * P:(k + 1) * P] for k in range(3)], axis=1)
    bcp = (b2.astype(np.float32) @ Wc.astype(np.float32) + bc).astype(np.float32)
    b1_full = np.tile(b1[None, :].astype(np.float32), (P, 1))
    bc_full = np.zeros((P, 8), np.float32)
    bc_full[:, :N_CLS] = bcp[None, :]

    # slot-ordered node data (slot s holds node n with perm[n] == s)
    xpad = np.zeros((NP_PAD, KP), bf16)
    xpad[perm[:N_NODES], :IN_F] = features.astype(bf16)
    inv_s_slot = np.ones(NP_PAD, np.float32)
    inv_d_slot = np.ones(NP_PAD, np.float32)
    inv_s_slot[perm] = inv_s
    inv_d_slot[perm] = inv_d

    iota128 = np.tile(np.arange(P, dtype=np.float32)[None, :], (P, 1)).astype(bf16)

    in_maps = []
    for c in range(NCORES):
        xt = np.ascontiguousarray(
            xpad[c * R:(c + 1) * R].reshape(RT, P, KC, P).transpose(0, 3, 2, 1)
        ).reshape(RT, P, KC * P)
        inv_s_tile = np.ascontiguousarray(inv_s_slot[c * R:(c + 1) * R].reshape(RT, P).T)
        inv_d_tile = np.ascontiguousarray(inv_d_slot[c * R:(c + 1) * R].reshape(RT, P).T)
        in_maps.append({
            "xT": xt,
            "w1": w1_sw,
            "w2c": w2c_sw,
            "b1t": b1_full,
            "bct": bc_full,
            "inv_s_t": inv_s_tile,
            "inv_d_t": inv_d_tile,
            "pcol": plan["pcol"][c],
            "iota": iota128,
            "idxs": plan["idx_wrapped"][c],
        })
    return plan, in_maps


def kernel(features, edge_src, edge_dst, W1, b1, W2, b2, Wc, bc):
    features = np.asarray(features, np.float32)
    edge_src = np.asarray(edge_src)
    edge_dst = np.asarray(edge_dst)
    plan, in_maps = _prepare(features, edge_src, edge_dst,
                             np.asarray(W1, np.float32), np.asarray(b1, np.float32),
                             np.asarray(W2, np.float32), np.asarray(b2, np.float32),
                             np.asarray(Wc, np.float32), np.asarray(bc, np.float32))
    nc = _build_nc(plan)

    from concourse.bass_utils import run_bass_kernel_spmd
    res = run_bass_kernel_spmd(nc, in_maps, core_ids=list(range(NCORES)))

    out_slots = np.zeros((NP_PAD, N_CLS), np.float32)
    for c in range(NCORES):
        buf = res.results[c]["out"]
        out_slots[c * R:(c + 1) * R] = (
            buf.reshape(P, RT, N_CLS).transpose(1, 0, 2).reshape(R, N_CLS))
    perm = plan["perm"]
    return out_slots[perm[:N_NODES]]
